# revision 1
# baseline (speedup 1.0000x reference)
"""Trainium2 Bass kernel for the DeepFuzzyCMean loss.

loss = GAMMA * sum_{n,k} u[n,k]^2 * ||x[n] - v[k]||^2
     = GAMMA * ( t1 + sum_k c[k]*|v_k|^2 - 2*sum_{k,d} W[k,d]*v[k,d] )
  t1 = sum_n (sum_k u2[n,k]) * |x_n|^2,  c = colsum(u2),  W = u2^T @ x

Device formulation (per 128-partition row tile, contraction over n in PSUM):
  xsq = [x*x | 1]  (x*x split across ACT Square / DVE mul / Pool mul so each
                    tile's squares finish inside the DMA arrival cadence;
                    ones cols preset once per buffer)
  MM1: acc_w[64,128]  += u2_pair^T @ x_pair    (W)      fp8 DoubleRow
  MM2: acc_q[64,129]  += u2_pair^T @ xsq_pair  ([t1|c]) fp8 DoubleRow
Host combines the per-core [64,257] partials with v in float64.

Wire format: x and u2=(u*USCALE)^2 cast to float8-e4m3 on host (memory-bound:
halves HBM traffic vs fp16; rel err ~5e-3). Shipping u pre-squared keeps ACT,
DVE and Pool free to split the x*x work. DoubleRow matmuls contract 256
rows/call (two adjacent row blocks per partition); pairing is consistent on
lhsT/rhs so the row-sum is unchanged. Consecutive rows map to one SBUF
partition so every DMA is a fully contiguous block (full line rate).

Tile sizes RAMP DOWN (`tiles`): big tiles amortize per-instruction overheads
mid-stream, and the tiny last tile keeps the final x-arrival -> square ->
matmul -> copy -> DMA-out chain short. PSUM->SBUF result copies run on ACT
and DVE in parallel.

Raw-bass implementation (manual semaphores, standalone sequencer waits).
Multi-buffered slots; data-parallel over N across 8 NeuronCores with a host
all-reduce.
"""

import sys
import types
from contextlib import ExitStack

import numpy as np
import ml_dtypes

import concourse.bass as bass
from concourse import mybir
from concourse.bass_utils import run_bass_kernel_spmd

# run_bass_kernel_spmd(trace=True) under axon imports antenv.axon_hooks,
# which this container lacks; stub it so a BASS_TRACE env var can't crash us.
try:
    import antenv.axon_hooks  # noqa: F401
except ImportError:
    try:
        import antenv

        _stub = types.ModuleType("antenv.axon_hooks")
        _stub.get_axon_ntff_profile_hook = lambda: None
        sys.modules["antenv.axon_hooks"] = _stub
        antenv.axon_hooks = _stub
    except ImportError:
        pass

GAMMA = 1e-06
N, K, D = 262144, 64, 128
NCORES = 8
N_CORE = N // NCORES
P = 128
OUT_W = 2 * D + 1  # [W | t1 block | c] = 257
USCALE = 64.0      # u pre-scale; partials carry USCALE^2 = 4096
F8NP = ml_dtypes.float8_e4m3

LAST_RESULTS = None
_NC_CACHE = {}

# model rates (ns) for the share balancer: per-block processing + fixed
# per-instruction overhead for each square engine
_RATE = {"act": (106.7, 185.0), "dve": (133.3, 60.0), "pool": (253.4, 95.0)}


def _balance(b):
    """Split b blocks across (act, dve, pool) minimizing the slowest engine;
    act/dve always get >= 1 block, pool may get 0 (it then emits no
    instruction for the tile and PE skips its wait)."""
    best, best_cost = None, None
    for a in range(1, b + 1):
        for v in range(1, b - a + 1):
            p = b - a - v
            if p < 0:
                continue
            cost = max(
                a * _RATE["act"][0] + _RATE["act"][1],
                v * _RATE["dve"][0] + _RATE["dve"][1],
                (p * _RATE["pool"][0] + _RATE["pool"][1]) if p else 0.0,
            )
            if best_cost is None or cost < best_cost:
                best, best_cost = (a, v, p), cost
    return best


def build_nc(
    n_rows=N_CORE,
    nbuf=None,
    num_devices=NCORES,
    reps=1,
    tiles=(46, 42, 38, 36, 32, 26, 16, 10, 6, 4),
    shares=None,
    schedule=None,
    shift=1,
    ship_xsq=1,
):
    """reps>1 repeats the full sweep inside one NEFF (re-reading the same
    DRAM) — used only for differential hardware timing.
    tiles = blocks (128 rows each) per iteration, all even, sum = n_rows/128.
    shares = per-tile (act, dve, pool) x*x block split; default rate-balanced.
    schedule = explicit DMA issue order: tokens ("x", i) / ("u", i); x tokens
    may run ahead of u tokens by at most nbuf-1 tiles (slot reuse bound).
    Default: x runs `shift` tiles ahead of u — squares chain off x arrivals
    while PE (the u consumer) trails anyway. nbuf=None caps slots at 6.
    ship_xsq = number of TRAILING tiles whose [x*x | 1] rows are shipped
    pre-squared from the host ("q" DMAs) instead of computed on device —
    removes the square stage from the final dependency chain at the cost of
    ~45ns of extra stream per block."""
    tiles = list(tiles) * reps
    iters = len(tiles)
    if reps > 1:
        ship_xsq = 0  # bench replay re-reads data; keep the simple path
    qset = set(range(iters - ship_xsq, iters)) if ship_xsq else set()
    if nbuf is None:
        # cap outstanding DMA issues: with nbuf == iters there are no
        # slot-free waits at all and the unthrottled issue burst overflows
        # the hardware DGE ring (NRT_EXEC_UNIT_UNRECOVERABLE); 6 keeps the
        # queue depth at the level the fp16/fp8 kernels ran reliably.
        nbuf = min(iters, 6)
    assert sum(tiles) * P == n_rows * reps
    assert all(b % 2 == 0 for b in tiles)
    assert iters >= nbuf
    if shares is None:
        shares = [_balance(b) for b in tiles]
    shares = [(0, 0, 0) if i in qset else s for i, s in enumerate(shares)]
    assert all(
        a + v + p == b
        for i, ((a, v, p), b) in enumerate(zip(shares, tiles))
        if i not in qset
    )
    t_max = max(tiles)
    data_iters = iters // reps
    f8 = mybir.dt.float8e4
    f32 = mybir.dt.float32
    w2 = D + 1
    # DRAM row offset of each tile (within one data pass; reps reuse them)
    roff = [0]
    for b in tiles[:data_iters]:
        roff.append(roff[-1] + b * P)

    q_rows = sum(tiles[i] for i in qset) * P
    qoff = {}
    qr = 0
    for i in sorted(qset):
        qoff[i] = qr
        qr += tiles[i] * P

    nc = bass.Bass("TRN2", num_devices=num_devices)
    x_d = nc.dram_tensor("x", [n_rows, D], f8, kind="ExternalInput")
    u_d = nc.dram_tensor("u", [n_rows, K], f8, kind="ExternalInput")
    qw = K + D + w2  # [u2 | x | x*x | 1] packed section-major per partition
    xq_d = (
        nc.dram_tensor("xq", [P, (q_rows // P) * qw], f8, kind="ExternalInput")
        if q_rows
        else None
    )
    out_d = nc.dram_tensor("out", [K, OUT_W], f32, kind="ExternalOutput")

    with ExitStack() as ctx:
        xt = [ctx.enter_context(nc.sbuf_tensor(f"xt{j}", [P, t_max * D], f8)) for j in range(nbuf)]
        u2 = [ctx.enter_context(nc.sbuf_tensor(f"u2{j}", [P, t_max * K], f8)) for j in range(nbuf)]
        xsq = [ctx.enter_context(nc.sbuf_tensor(f"xsq{j}", [P, t_max * w2], f8)) for j in range(nbuf)]
        q_blocks = max((tiles[i] for i in qset), default=0)
        xc = (
            ctx.enter_context(nc.sbuf_tensor("xc", [P, q_blocks * qw], f8))
            if qset
            else None
        )
        res = ctx.enter_context(nc.sbuf_tensor("res", [K, OUT_W], f32))
        acc_w = ctx.enter_context(nc.psum_tensor([K, D], f32))
        acc_q = ctx.enter_context(nc.psum_tensor([K, w2], f32))

        s_dx = [ctx.enter_context(nc.semaphore(f"s_dx{j}")) for j in range(nbuf)]
        s_q = {i: ctx.enter_context(nc.semaphore(f"s_q{i}")) for i in sorted(qset)}
        s_du = [ctx.enter_context(nc.semaphore(f"s_du{j}")) for j in range(nbuf)]
        s_act = ctx.enter_context(nc.semaphore("s_act"))
        s_dve = ctx.enter_context(nc.semaphore("s_dve"))
        s_pool = ctx.enter_context(nc.semaphore("s_pool"))
        s_pe = ctx.enter_context(nc.semaphore("s_pe"))
        s_res = ctx.enter_context(nc.semaphore("s_res"))
        s_do = ctx.enter_context(nc.semaphore("s_do"))

        block = ctx.enter_context(nc.Block())

        def views(j):
            xsq3 = xsq[j][:, :].rearrange("p (b c) -> p b c", b=t_max)
            xt3 = xt[j][:, :].rearrange("p (b d) -> p b d", b=t_max)
            return xsq3, xt3

        def tile_src(i):
            r = roff[i % data_iters]
            b = tiles[i]
            x_src = x_d[r : r + P * b, :].rearrange("(p b) d -> p (b d)", p=P)
            u_src = u_d[r : r + P * b, :].rearrange("(p b) k -> p (b k)", p=P)
            return x_src, u_src

        if schedule is None:
            schedule = []
            xi = ui = 0
            while xi < iters or ui < iters:
                if xi < iters and (xi - ui < shift + 1):
                    if xi not in qset:
                        schedule.append(("x", xi))
                    xi += 1
                else:
                    if ui not in qset:
                        schedule.append(("u", ui))
                    ui += 1
            schedule += [("q", i) for i in sorted(qset)]

        # Static timing model of the serial DMA stream + square-engine
        # chains: used only to ORDER the PE's per-tile semaphore waits so
        # the predicted laggard pays a single decode.
        x_ns = [max(b * 128 * (2.0 if b * 128 < 512 else 1.0) / 22.5, 7) * 8 for b in tiles]
        u_ns = [max(b * 64 * (2.0 if b * 64 < 512 else 1.0) / 22.5, 7) * 8 for b in tiles]
        q_ns = [max(b * qw * (2.0 if b * qw < 512 else 1.0) / 22.5, 7) * 8 for b in tiles]
        tpos = 2332.0
        arr_x = [0.0] * iters
        for kind, i in schedule:
            tpos += {"x": x_ns, "u": u_ns, "q": q_ns}[kind][i]
            if kind == "x":
                arr_x[i] = tpos + 900.0
        eng_end = {"act": 0.0, "dve": 0.0, "pool": 0.0}
        wait_order = []
        for i in range(iters):
            used = []
            for e, sh in zip(("act", "dve", "pool"), shares[i]):
                if sh:
                    eng_end[e] = max(eng_end[e], arr_x[i]) + sh * _RATE[e][0] + _RATE[e][1]
                    used.append(e)
            wait_order.append(sorted(used, key=lambda e: eng_end[e]))
        # per-engine square-sem increments completed through tile i
        # (engines skip zero-share tiles, e.g. shipped-xsq ones)
        acnt, vcnt, pcnt = [], [], []
        ca = cv = cp = 0
        for i in range(iters):
            ca += 1 if shares[i][0] else 0
            cv += 1 if shares[i][1] else 0
            cp += 1 if shares[i][2] else 0
            acnt.append(ca)
            vcnt.append(cv)
            pcnt.append(cp)

        @block.sync
        def _(sync):
            sched = schedule

            def slot_free_waits(i):
                # slot j free: PE done with xt/u2/xsq, the square engines done
                # with xt of iteration i-nbuf; the slot's own previous DMAs
                # completed long ago (consumers saw them) — these waits are
                # usually already satisfied but keep the per-sem increments
                # ordered for the race checker.
                j = i % nbuf
                if i >= nbuf:
                    k = i - nbuf
                    sync.wait_ge(s_pe, k + 1)
                    if shares[k][1]:
                        sync.wait_ge(s_dve, vcnt[k])
                    if shares[k][0]:
                        sync.wait_ge(s_act, acnt[k])
                    if shares[k][2]:
                        sync.wait_ge(s_pool, pcnt[k])
                    sync.wait_ge(s_dx[j], 16 * (i // nbuf))
                    sync.wait_ge(s_du[j], 16 * (i // nbuf))

            for kind, i in sched:
                j = i % nbuf
                b = tiles[i]
                x_src, u_src = tile_src(i)
                if kind == "x":
                    slot_free_waits(i)
                    sync.dma_start(out=xt[j][:, 0 : b * D], in_=x_src).then_inc(
                        s_dx[j], 16
                    )
                elif kind == "u":
                    sync.dma_start(out=u2[j][:, 0 : b * K], in_=u_src).then_inc(
                        s_du[j], 16
                    )
                else:  # "q": packed section-major shipped tile -> xc
                    o = (qoff[i] // P) * qw
                    sync.dma_start(
                        out=xc[:, 0 : b * qw], in_=xq_d[:, o : o + b * qw]
                    ).then_inc(s_q[i], 16)
            sync.wait_ge(s_res, 2)
            sync.dma_start(out=out_d[:, :], in_=res[:, :]).then_inc(s_do, 16)
            sync.wait_ge(s_do, 16)

        @block.scalar
        def _(scalar):
            # ACT squares the first share of each tile.
            for i in range(iters):
                j = i % nbuf
                a = shares[i][0]
                if not a:
                    continue
                if i >= nbuf:
                    scalar.wait_ge(s_pe, i - nbuf + 1)  # xsq slot reader
                scalar.wait_ge(s_dx[j], 16 * (i // nbuf + 1))
                xsq3, xt3 = views(j)
                scalar.activation(
                    out=xsq3[:, 0:a, 0:D],
                    in_=xt3[:, 0:a, :],
                    func=mybir.ActivationFunctionType.Square,
                ).then_inc(s_act)
            # tail: psum -> sbuf (acc_q goes via DVE in parallel) ->
            # (sync engine DMAs res out)
            scalar.wait_ge(s_pe, iters)
            scalar.copy(res[:, 0:D], acc_w[:, :]).then_inc(s_res)

        @block.vector
        def _(vector):
            # ones columns are static: set once per buffer, never overwritten
            for j in range(nbuf):
                xsq3, _ = views(j)
                vector.memset(xsq3[:, :, D : D + 1], 1.0)
            # DVE squares the middle share of each tile
            for i in range(iters):
                j = i % nbuf
                a, v, _ = shares[i]
                if not v:
                    continue
                if i >= nbuf:
                    vector.wait_ge(s_pe, i - nbuf + 1)  # xsq slot reader
                vector.wait_ge(s_dx[j], 16 * (i // nbuf + 1))
                xsq3, xt3 = views(j)
                vector.tensor_mul(
                    xsq3[:, a : a + v, 0:D],
                    xt3[:, a : a + v, :],
                    xt3[:, a : a + v, :],
                ).then_inc(s_dve)
            # parallel tail copy: acc_q -> res while ACT copies acc_w
            vector.wait_ge(s_pe, iters)
            vector.tensor_copy(res[:, D:OUT_W], acc_q[:, :]).then_inc(s_res)

        @block.gpsimd
        def _(gp):
            # Pool squares the trailing share of each tile.
            for i in range(iters):
                j = i % nbuf
                a, v, p = shares[i]
                b = tiles[i]
                if not p:
                    continue
                if i >= nbuf:
                    gp.wait_ge(s_pe, i - nbuf + 1)  # xsq slot reader
                gp.wait_ge(s_dx[j], 16 * (i // nbuf + 1))
                xsq3, xt3 = views(j)
                gp.tensor_mul(
                    xsq3[:, a + v : b, 0:D],
                    xt3[:, a + v : b, :],
                    xt3[:, a + v : b, :],
                ).then_inc(s_pool)

        @block.tensor
        def _(tensor):
            for i in range(iters):
                j = i % nbuf
                b = tiles[i]
                # x-arrival is implied by the square sems (the three square
                # engines jointly read every x block first); wait order:
                # earliest-predicted-done first so the laggard pays one
                # decode.
                sq_sem = {"act": s_act, "dve": s_dve, "pool": s_pool}
                sq_val = {"act": acnt[i], "dve": vcnt[i], "pool": pcnt[i]}
                if i in qset:
                    # packed shipped tile: one DMA, one wait; section-major
                    # layout keeps every matmul view a contiguous-pair slice
                    # with the same AP structure as the normal path
                    tensor.wait_ge(s_q[i], 16)
                    xoff, qoff2 = b * K, b * (K + D)
                else:
                    tensor.wait_ge(s_du[j], 16 * (i // nbuf + 1))
                    for e in wait_order[i]:
                        tensor.wait_ge(sq_sem[e], sq_val[e])
                last = None
                for bb in range(b // 2):
                    if i in qset:
                        lhsT = xc[:, 2 * bb * K : (2 * bb + 2) * K].rearrange(
                            "p (two k) -> p two k", two=2
                        )
                        rhs_w = xc[
                            :, xoff + 2 * bb * D : xoff + (2 * bb + 2) * D
                        ].rearrange("p (two d) -> p two d", two=2)
                        rhs_q = xc[
                            :, qoff2 + 2 * bb * w2 : qoff2 + (2 * bb + 2) * w2
                        ].rearrange("p (two c) -> p two c", two=2)
                    else:
                        lhsT = u2[j][:, 2 * bb * K : (2 * bb + 2) * K].rearrange(
                            "p (two k) -> p two k", two=2
                        )
                        rhs_w = xt[j][:, 2 * bb * D : (2 * bb + 2) * D].rearrange(
                            "p (two d) -> p two d", two=2
                        )
                        rhs_q = xsq[j][:, 2 * bb * w2 : (2 * bb + 2) * w2].rearrange(
                            "p (two c) -> p two c", two=2
                        )
                    tensor.matmul(
                        acc_w[:, :],
                        lhsT=lhsT,
                        rhs=rhs_w,
                        start=(i == 0 and bb == 0),
                        stop=(i == iters - 1 and bb == b // 2 - 1),
                        perf_mode=mybir.MatmulPerfMode.DoubleRow,
                    )
                    last = tensor.matmul(
                        acc_q[:, :],
                        lhsT=lhsT,
                        rhs=rhs_q,
                        start=(i == 0 and bb == 0),
                        stop=(i == iters - 1 and bb == b // 2 - 1),
                        perf_mode=mybir.MatmulPerfMode.DoubleRow,
                    )
                last.then_inc(s_pe)

    return nc


def combine_host(parts, v):
    """Combine per-core [K, OUT_W] partials (scaled by USCALE^2) with v in
    float64 on the host."""
    acc = np.zeros((K, OUT_W), np.float64)
    for p in parts:
        acc += np.asarray(p, np.float64)
    acc /= USCALE * USCALE
    W = acc[:, :D]
    t1 = acc[:, D : 2 * D].sum()
    c = acc[:, 2 * D]
    v64 = np.asarray(v, np.float64)
    v2 = (v64 * v64).sum(axis=1)
    loss = t1 + (v2 * c).sum() - 2.0 * (W * v64).sum()
    return np.asarray(GAMMA * loss, dtype=np.float32)


# rows per core whose [x*x | 1] ships pre-squared (the trailing 4-block
# tile of the default ramp)
_Q_ROWS = 4 * P


def kernel(x, u, v):
    global LAST_RESULTS
    x = np.asarray(x)
    u = np.asarray(u)
    assert x.shape == (N, D) and u.shape == (N, K)
    x8 = np.ascontiguousarray(x.astype(F8NP))
    u32 = np.asarray(u, np.float32) * USCALE
    u28 = np.ascontiguousarray((u32 * u32).astype(F8NP))

    if "nc" not in _NC_CACHE:
        _NC_CACHE["nc"] = build_nc()
    nc = _NC_CACHE["nc"]

    in_maps = []
    for c in range(NCORES):
        xc = x8[c * N_CORE : (c + 1) * N_CORE]
        uc = u28[c * N_CORE : (c + 1) * N_CORE]
        # packed shipped tail, section-major per partition:
        # [u2 blocks | x blocks | [x*x|1] blocks]; squares the SAME fp8
        # values the device would square
        nb = _Q_ROWS // P
        xt8 = xc[N_CORE - _Q_ROWS :]
        xt32 = np.asarray(xt8, np.float32)
        xsq8 = np.ones((_Q_ROWS, D + 1), F8NP)
        xsq8[:, :D] = (xt32 * xt32).astype(F8NP)
        xq = np.concatenate(
            [
                uc[N_CORE - _Q_ROWS :].reshape(P, nb * K),
                xt8.reshape(P, nb * D),
                xsq8.reshape(P, nb * (D + 1)),
            ],
            axis=1,
        )
        in_maps.append({"x": xc, "u": uc, "xq": np.ascontiguousarray(xq)})
    LAST_RESULTS = run_bass_kernel_spmd(nc, in_maps, list(range(NCORES)))
    return combine_host([r["out"] for r in LAST_RESULTS.results], v)



# revision 11
# speedup vs baseline: 1.0335x; 1.0335x over previous
"""Trainium2 Bass kernel for the DeepFuzzyCMean loss.

loss = GAMMA * sum_{n,k} u[n,k]^2 * ||x[n] - v[k]||^2
     = GAMMA * ( sum_k t1_k + sum_k c_k*|v_k|^2 - 2*sum_{k,d} W[k,d]*v[k,d] )
  W    = u2^T @ x          [K, D]
  t1_k = sum_n u2[n,k]*r_n    with r_n = |x_n|^2
  c_k  = sum_n u2[n,k]

Device formulation: ship per row the fp8 record [u2 (64B) | x (128B) | r | 1]
(194 B/row; r = fp8(|x_n|^2) computed host-side in fp32, "1" a literal ones
byte). ONE DoubleRow fp8 matmul per row-pair then produces all three terms at
once:  acc[64, 130] += u2_pair^T @ [x | r | 1]_pair.  No on-device squaring
pipeline at all -- the memory stream (194 B/row at ~360 GB/s/core) is the only
real cost; PE trails at ~25% duty.

Tail: the [64,130] fp32 result leaves PSUM via a parallel ACT/DVE copy into
SBUF, then a *pre-prepared* SWDGE dma_scatter_add (descriptors generated
mid-stream on the Pool queue, identity indices via iota) is fired with a
cheap trigger_dma -- skipping the ~1.3us HWDGE descriptor-gen + DGE delay
that a plain DMACopy would put on the critical path. The DRAM target is
pre-zeroed by a small hidden DMA early in the stream (and bass2jax pre-zeros
ExternalOutput buffers anyway), so scatter-ADD == plain write.

Tile sizes ramp down so the final x-arrival -> matmul -> copy -> trigger
chain is short. Raw-bass (manual semaphores); data-parallel over N across 8
NeuronCores with a host all-reduce of the per-core [64, 192] partials.
"""

import sys
import types
from contextlib import ExitStack

import numpy as np
import ml_dtypes

import concourse.bass as bass
from concourse import mybir
from concourse.bass_utils import run_bass_kernel_spmd

# run_bass_kernel_spmd(trace=True) under axon imports antenv.axon_hooks,
# which this container lacks; stub it so a BASS_TRACE env var can't crash us.
try:
    import antenv.axon_hooks  # noqa: F401
except ImportError:
    try:
        import antenv

        _stub = types.ModuleType("antenv.axon_hooks")
        _stub.get_axon_ntff_profile_hook = lambda: None
        sys.modules["antenv.axon_hooks"] = _stub
        antenv.axon_hooks = _stub
    except ImportError:
        pass

GAMMA = 1e-06
N, K, D = 262144, 64, 128
NCORES = 8
N_CORE = N // NCORES
P = 128
XRW = D + 2        # [x | r | 1] record width = 130
RW = K + XRW       # full packed row = 194 bytes
OUT_W = XRW        # live output cols: [W | t1 | c] = 130
OUT_PAD = 192      # padded out row (fp32) so the scatter stride is 768B (%256)
USCALE = 64.0      # u pre-scale; partials carry USCALE^2 = 4096
CSPL = 20          # ACT/DVE copy split column (ACT is slower per column)
CSPL2 = 130        # DVE/Pool copy split column (130 = Pool copy disabled)
TILES = (46, 46, 46, 40, 32, 22, 14, 6, 4)  # blocks/tile, sum 256, ramp down
F8NP = ml_dtypes.float8_e4m3

LAST_RESULTS = None
_NC_CACHE = {}


def build_nc(
    n_rows=N_CORE,
    nbuf=6,
    num_devices=NCORES,
    reps=1,
    tiles=TILES,
    zero_out=True,
    scatter_out=False,
    final_wait=False,
    out_wait_res=True,
    prep_only_probe=False,
):
    """tiles = blocks (128 rows each) per iteration, all even, sum = n_rows/128.
    reps>1 repeats the sweep inside one NEFF re-reading the same DRAM (timing
    only; the PSUM result is then reps*the real one). nbuf caps outstanding
    DMA issues (hardware DGE ring depth ran reliably at 6). zero_out ships an
    extra early DMA that zeroes the scatter target (bass2jax pre-zeros
    ExternalOutput buffers too; this is belt-and-braces). scatter_out=False
    falls back to a plain SP DMACopy for the result (slower tail)."""
    tiles = list(tiles) * reps
    iters = len(tiles)
    assert sum(tiles) * P == n_rows * reps
    assert all(b % 2 == 0 for b in tiles)
    assert iters >= nbuf
    t_max = max(tiles)
    data_iters = iters // reps
    f8 = mybir.dt.float8e4
    f32 = mybir.dt.float32
    # free-dim byte offset of each tile in the packed xu tensor (one pass)
    boff = [0]
    for b in tiles[:data_iters]:
        boff.append(boff[-1] + b * RW)

    nc = bass.Bass("TRN2", num_devices=num_devices)
    xu_d = nc.dram_tensor("xu", [P, (n_rows // P) * RW], f8, kind="ExternalInput")
    out_d = nc.dram_tensor("out", [K, OUT_PAD], f32, kind="ExternalOutput")

    with ExitStack() as ctx:
        slot = [
            ctx.enter_context(nc.sbuf_tensor(f"sl{j}", [P, t_max * RW], f8))
            for j in range(nbuf)
        ]
        res = ctx.enter_context(nc.sbuf_tensor("res", [P, OUT_PAD], f32))
        idxs = ctx.enter_context(nc.sbuf_tensor("idxs", [P, K // 16], mybir.dt.int16))
        acc = ctx.enter_context(nc.psum_tensor([K, XRW], f32))

        s_d = [ctx.enter_context(nc.semaphore(f"s_d{j}")) for j in range(nbuf)]
        s_pe = ctx.enter_context(nc.semaphore("s_pe"))
        s_rz = ctx.enter_context(nc.semaphore("s_rz"))
        s_z = ctx.enter_context(nc.semaphore("s_z"))
        s_idx = ctx.enter_context(nc.semaphore("s_idx"))
        s_prep = ctx.enter_context(nc.semaphore("s_prep"))
        s_res = ctx.enter_context(nc.semaphore("s_res"))
        s_do = ctx.enter_context(nc.semaphore("s_do"))

        block = ctx.enter_context(nc.Block())

        @block.sync
        def _(sync):
            for i in range(iters):
                j = i % nbuf
                b = tiles[i]
                o = boff[i % data_iters]
                if i >= nbuf:
                    # slot j free: PE consumed it, and its own previous DMA
                    # long completed (keeps per-sem increments ordered)
                    sync.wait_ge(s_pe, i - nbuf + 1)
                    sync.wait_ge(s_d[j], 16 * (i // nbuf))
                sync.dma_start(
                    out=slot[j][:, 0 : b * RW], in_=xu_d[:, o : o + b * RW]
                ).then_inc(s_d[j], 16)
                if i == 0 and zero_out and scatter_out:
                    # hidden early zeroing of the scatter target, reading the
                    # freshly-memset res buffer
                    sync.wait_ge(s_rz, 1)
                    sync.dma_start(out=out_d[:, :], in_=res[0:K, :]).then_inc(
                        s_z, 16
                    )
            if not scatter_out:
                if out_wait_res:
                    sync.wait_ge(s_res, 3 if CSPL2 < OUT_W else 2)
                else:
                    sync.wait_ge(s_pe, iters)
                sync.dma_start(
                    out=out_d[:, 0:OUT_W], in_=res[0:K, 0:OUT_W]
                ).then_inc(s_do, 16)
                if final_wait:
                    sync.wait_ge(s_do, 16)

        @block.tensor
        def _(tensor):
            for i in range(iters):
                j = i % nbuf
                b = tiles[i]
                tensor.wait_ge(s_d[j], 16 * (i // nbuf + 1))
                xoff = b * K
                last = None
                for bb in range(b // 2):
                    lhsT = slot[j][:, 2 * bb * K : (2 * bb + 2) * K].rearrange(
                        "p (two k) -> p two k", two=2
                    )
                    rhs = slot[j][
                        :, xoff + 2 * bb * XRW : xoff + (2 * bb + 2) * XRW
                    ].rearrange("p (two c) -> p two c", two=2)
                    last = tensor.matmul(
                        acc[:, :],
                        lhsT=lhsT,
                        rhs=rhs,
                        start=(i == 0 and bb == 0),
                        stop=(i == iters - 1 and bb == b // 2 - 1),
                        perf_mode=mybir.MatmulPerfMode.DoubleRow,
                    )
                last.then_inc(s_pe)

        @block.vector
        def _(vector):
            if scatter_out:
                # res doubles as the zero source for the early out-zeroing
                # DMA; cols OUT_W:OUT_PAD stay zero (the scatter reads 192).
                vector.memset(res[:, :], 0.0).then_inc(s_rz)
            vector.wait_ge(s_pe, iters)
            if zero_out and scatter_out:
                vector.wait_ge(s_z, 16)  # WAR vs the zeroing DMA's read
            vector.tensor_copy(res[0:K, CSPL:CSPL2], acc[:, CSPL:CSPL2]).then_inc(s_res)

        @block.scalar
        def _(scalar):
            scalar.wait_ge(s_pe, iters)
            if zero_out and scatter_out:
                scalar.wait_ge(s_z, 16)
            scalar.copy(res[0:K, 0:CSPL], acc[:, 0:CSPL]).then_inc(s_res)

        @block.gpsimd
        def _(gp):
            if not scatter_out and not prep_only_probe:
                if CSPL2 < OUT_W:
                    gp.wait_ge(s_pe, iters)
                    gp.tensor_copy(res[0:K, CSPL2:OUT_W], acc[:, CSPL2:OUT_W]).then_inc(s_res)
                return
            # identity scatter indices: token t (partition t%16, slot t//16)
            # -> out row t; partitions 16.. get -1 (ignored, keeps the
            # executor's range assert happy)
            gp.memset(idxs[:, :], -1)
            gp.iota(
                idxs[0:16, :], pattern=[[16, K // 16]], base=0, channel_multiplier=1
            ).then_inc(s_idx)
            gp.wait_ge(s_idx, 1)
            # descriptor prep happens HERE (mid-stream, off the critical
            # path); the DMA fires at trigger_dma below
            gp.dma_scatter_add(
                out_d[:, :].rearrange("k (one w) -> k one w", one=1),
                res[:, :].rearrange("p (one w) -> p one w", one=1),
                idxs[:, :],
                K,
                K,
                OUT_PAD,
                prepare_only=True,
                sem=s_do,
            ).then_inc(s_prep, 1)
            gp.wait_ge(s_prep, 1)
            if prep_only_probe:
                return  # leave the descriptor untriggered; out comes via DMACopy
            if zero_out:
                gp.wait_ge(s_z, 16)
            gp.wait_ge(s_res, 2)
            gp.trigger_dma(count=1)
            gp.wait_ge(s_do, 16)

    if scatter_out:
        _fill_trigger_isa_bytes(nc)
    return nc


def _fill_trigger_isa_bytes(nc):
    """bass's InstTriggerDma serializes with empty `instr` bytes (its encoding
    normally happens in bass-native codegen), which walrus codegen rejects
    ("ISA wrong length"). Fill in the 64-byte TRIGGER_DMA encoding from this
    container's ISA table so walrus can pass it through; the sim still
    dispatches on the InstTriggerDma type, so its SWDGE-drain timing/exec
    semantics are unchanged."""
    from concourse import bass_isa

    op = nc.isa.Opcode.NEURON_ISA_TPB_OPCODE_TRIGGER_DMA
    for blk in nc.m.functions[0].blocks:
        for inst in blk.instructions:
            if type(inst).__name__ == "InstTriggerDma":
                instr, _fix = bass_isa.isa_struct(
                    nc.isa,
                    op,
                    {
                        "count": inst._count,
                        "count_is_reg": 0,
                        "queue_num": inst.queue_num,
                    },
                )
                inst.instr = instr
                inst.isa_opcode = op.value


def pack_core(x8, u28, r8):
    """Pack one core's rows into the [P, rows/P * RW] fp8 wire tensor.

    Per tile of b blocks starting at row r0, partition p holds rows
    r0 + p*b .. r0 + p*b + b - 1: first the b u2 records (64B each), then the
    b [x | r | 1] records (130B each)."""
    rows = x8.shape[0]
    ones = np.ones((rows, 1), F8NP)
    xr = np.concatenate([x8, r8, ones], axis=1)  # [rows, 130]
    out = np.empty((P, (rows // P) * RW), F8NP)
    off = 0
    r0 = 0
    for b in TILES:
        nr = b * P
        u_t = u28[r0 : r0 + nr].reshape(P, b * K)
        x_t = xr[r0 : r0 + nr].reshape(P, b * XRW)
        out[:, off : off + b * K] = u_t
        out[:, off + b * K : off + b * RW] = x_t
        off += b * RW
        r0 += nr
    assert r0 == rows and off == out.shape[1]
    return out


def combine_host(parts, v):
    """Combine per-core [K, OUT_PAD] partials (scaled by USCALE^2) with v in
    float64 on the host."""
    acc = np.zeros((K, OUT_W), np.float64)
    for p in parts:
        acc += np.asarray(p, np.float64)[:, :OUT_W]
    acc /= USCALE * USCALE
    W = acc[:, :D]
    t1 = acc[:, D].sum()
    c = acc[:, D + 1]
    v64 = np.asarray(v, np.float64)
    v2 = (v64 * v64).sum(axis=1)
    loss = t1 + (v2 * c).sum() - 2.0 * (W * v64).sum()
    return np.asarray(GAMMA * loss, dtype=np.float32)


def kernel(x, u, v):
    global LAST_RESULTS
    x = np.asarray(x, np.float32)
    u = np.asarray(u, np.float32)
    assert x.shape == (N, D) and u.shape == (N, K)
    x8 = np.ascontiguousarray(x.astype(F8NP))
    u32 = u * USCALE
    u28 = np.ascontiguousarray((u32 * u32).astype(F8NP))
    # per-row |x|^2 in fp32, clamped under the fp8-e4m3 finite max
    r = np.minimum((x * x).sum(axis=1, keepdims=True), 224.0)
    r8 = r.astype(F8NP)

    if "nc" not in _NC_CACHE:
        _NC_CACHE["nc"] = build_nc()
    nc = _NC_CACHE["nc"]

    in_maps = []
    for c in range(NCORES):
        sl = slice(c * N_CORE, (c + 1) * N_CORE)
        in_maps.append({"xu": pack_core(x8[sl], u28[sl], r8[sl])})
    LAST_RESULTS = run_bass_kernel_spmd(nc, in_maps, list(range(NCORES)))
    return combine_host([r_["out"] for r_ in LAST_RESULTS.results], v)
